# revision 12
# baseline (speedup 1.0000x reference)
"""BioWaveKAN fused kernel for 8 Trainium2 NeuronCores.

y = wavelet(x) @ (pi^-1/4 * Ww).T + x @ (0.3 * Wb).T   (single K=4096 contraction)
out = BatchNorm1d(y)  (training-mode batch stats, all-reduced across cores)

Sharding: data-parallel over batch (8 x 512 rows); BN stats via four 4KB
AllReduces, the first three hidden under the matmuls. Device layout is
transposed (features on partitions); host pre-transposes x and post-transposes
the output. Matmuls run fp16 x fp16 (fp32 PSUM accumulate); wavelet math runs
fp32 on ACT/DVE with an exact magic-number range reduction for cos(3u).

Structure per core:
  pass 1 (k-tiles 0..15  = x):       psum -> y_partial (ACT copy)
  pass 2 (k-tiles 16..31 = wavelet): y = y_partial + psum (DVE, fused sum(y))
                                     + DVE tensor_tensor_reduce (sum(y^2))
"""
import math

import numpy as np

from concourse import bacc
import concourse.tile as tile
import concourse.mybir as mybir
from concourse.bass_utils import run_bass_kernel_spmd

F32 = mybir.dt.float32
F16 = mybir.dt.float16
AF = mybir.ActivationFunctionType
OP = mybir.AluOpType

B = 4096          # batch
D = 2048          # in_dim == out_dim
NCORES = 8
BS = B // NCORES  # batch shard per core (512)
NIT = D // 128    # i-tiles (16)
NKT = 2 * NIT     # contraction tiles (32): 0..15 = x, 16..31 = wavelet
NOT = D // 128    # o-tiles (16)
NQ = 4            # quarters of o-tiles
BN_EPS = 1e-5
TWO_PI = 2.0 * math.pi
MAGIC = 1.5 * 2.0 ** 23

_CACHE = {}


def _build_nc():
    nc = bacc.Bacc()

    xT_d = nc.dram_tensor("xT", (D, BS), F16, kind="ExternalInput")
    wT_d = nc.dram_tensor("wT", (2 * D, D), F16, kind="ExternalInput")
    s3_d = nc.dram_tensor("s3", (128, NIT), F32, kind="ExternalInput")
    b3_d = nc.dram_tensor("b3", (128, NIT), F32, kind="ExternalInput")
    su_d = nc.dram_tensor("su", (128, NIT), F32, kind="ExternalInput")
    bu_d = nc.dram_tensor("bu", (128, NIT), F32, kind="ExternalInput")
    gm_d = nc.dram_tensor("gm", (128, NOT), F32, kind="ExternalInput")
    bt_d = nc.dram_tensor("bt", (128, NOT), F32, kind="ExternalInput")

    yT_d = nc.dram_tensor("yT", (D, BS), F32, kind="ExternalOutput")

    xT_t = xT_d[:].rearrange("(kt p) b -> p kt b", p=128)      # [128, 16, BS]
    wT_t = wT_d[:].rearrange("(kt p) o -> p kt o", p=128)      # [128, 32, D]
    yT_t = yT_d[:].rearrange("(mt p) b -> p mt b", p=128)      # [128, 16, BS]

    # stats column layout: quarter q holds cols [8q, 8q+8):
    #   [8q + ml]     = sum(y)   for o-tile m = 4q + ml
    #   [8q + 4 + ml] = sum(y^2)
    with tile.TileContext(nc) as tc:
        with (
            tc.tile_pool(name="big", bufs=1) as big,
            tc.tile_pool(name="small", bufs=1) as small,
            tc.tile_pool(name="wqh", bufs=2) as wqh,
            tc.tile_pool(name="wq", bufs=3) as wq,
            tc.tile_pool(name="scr", bufs=6) as scr,
            tc.tile_pool(name="escr", bufs=3) as escr,
            tc.tile_pool(name="drscr", bufs=3) as drscr,
            tc.tile_pool(name="ps", bufs=8, space="PSUM") as ps,
            tc.tile_pool(name="dram", bufs=1, space="DRAM") as dram,
        ):
            rhs = big.tile([128, NKT, BS], F16)

            # ---- DMA issue order: x chunk 0, first weight (split), consts,
            #      rest of x, remaining weights ----
            nc.sync.dma_start(rhs[:, 0:4, :], xT_t[:, 0:4, :])

            wtiles = {}
            w00a0 = wqh.tile([128, 4, 512], F16, tag="wqha", name="w00a0")
            nc.sync.dma_start(w00a0[:], wT_t[:, 0:4, 0:512])
            w00a1 = wqh.tile([128, 4, 512], F16, tag="wqha", name="w00a1")
            nc.sync.dma_start(w00a1[:], wT_t[:, 4:8, 0:512])
            w00b = wqh.tile([128, 8, 512], F16, tag="wqh", name="w00b")
            nc.sync.dma_start(w00b[:], wT_t[:, 8:16, 0:512])

            w01 = wq.tile([128, NIT, 512], F16, tag="wq", name="w_0_1")
            nc.sync.dma_start(w01[:], wT_t[:, 0:NIT, 512:1024])
            wtiles[(0, 1)] = w01

            s3t = small.tile([128, NIT], F32)
            b3t = small.tile([128, NIT], F32)
            sut = small.tile([128, NIT], F32)
            but = small.tile([128, NIT], F32)
            gmt = small.tile([128, NOT], F32)
            btt = small.tile([128, NOT], F32)
            nc.sync.dma_start(s3t[:], s3_d[:])
            nc.sync.dma_start(b3t[:], b3_d[:])
            nc.sync.dma_start(sut[:], su_d[:])
            nc.sync.dma_start(but[:], bu_d[:])
            nc.sync.dma_start(gmt[:], gm_d[:])
            nc.sync.dma_start(btt[:], bt_d[:])

            for c in range(1, 4):
                nc.sync.dma_start(rhs[:, c * 4:(c + 1) * 4, :],
                                  xT_t[:, c * 4:(c + 1) * 4, :])

            for h in range(2):
                for q in range(NQ):
                    if (h, q) in ((0, 0), (0, 1)):
                        continue
                    wt = wq.tile([128, NIT, 512], F16, tag="wq",
                                 name=f"w_{h}_{q}")
                    nc.sync.dma_start(
                        wt[:],
                        wT_t[:, h * NIT:(h + 1) * NIT, q * 512:(q + 1) * 512])
                    wtiles[(h, q)] = wt

            magict = small.tile([128, 1], F32)
            nc.vector.memset(magict[:], MAGIC)
            zbt = small.tile([128, 1], F32)
            nc.vector.memset(zbt[:], 0.0)
            epst = small.tile([128, 1], F32)
            nc.vector.memset(epst[:], BN_EPS)

            # ---- wavelet phase A: sin tiles (trig table set) ----
            # t/r on DVE, the magic-number round on GpSimd (parallel engines)
            for i in range(NIT):
                xf = rhs[:, i, :]
                tt = scr.tile([128, BS], F32, tag="scr", name=f"t_{i}")
                nc.vector.tensor_scalar(out=tt[:], in0=xf,
                                        scalar1=s3t[:, i:i + 1],
                                        scalar2=b3t[:, i:i + 1],
                                        op0=OP.mult, op1=OP.add)
                kt_ = scr.tile([128, BS], F32, tag="scr", name=f"k_{i}")
                nc.vector.tensor_scalar(out=kt_[:], in0=tt[:],
                                        scalar1=magict[:], scalar2=magict[:],
                                        op0=OP.add, op1=OP.subtract)
                rt = scr.tile([128, BS], F32, tag="scr", name=f"r_{i}")
                nc.vector.tensor_tensor(rt[:], tt[:], kt_[:], op=OP.subtract)
                nc.scalar.activation(rhs[:, NIT + i, :], rt[:], AF.Sin,
                                     bias=zbt[:], scale=TWO_PI)

            # ---- wavelet phase B: exp tiles (exp table set) ----
            for i in range(NIT):
                xf = rhs[:, i, :]
                qt = scr.tile([128, BS], F32, tag="scr", name=f"qq_{i}")
                nc.scalar.activation(qt[:], xf, AF.Square,
                                     bias=but[:, i:i + 1], scale=sut[:, i:i + 1])
                et = escr.tile([128, BS], F16, tag="escr", name=f"e_{i}")
                nc.scalar.activation(et[:], qt[:], AF.Exp,
                                     bias=zbt[:], scale=-0.5)
                nc.vector.tensor_tensor(rhs[:, NIT + i, :],
                                        rhs[:, NIT + i, :], et[:],
                                        op=OP.mult)

            # prefetch the sqrt table set (Copy lives in every set, so the h0
            # drains below don't reload; the finalize Sqrts are then free)
            sqpre = small.tile([128, 1], F32)
            nc.scalar.activation(sqpre[:], zbt[:], AF.Sqrt, bias=epst[:])

            # ---- matmuls + fused drains + per-quarter stats AllReduce ----
            y_big = big.tile([128, NOT, BS], F32)
            stats = small.tile([128, 2 * NOT], F32)
            red = small.tile([128, 2 * NOT], F32)
            ab = small.tile([128, 2 * NOT], F32)   # A cols 0..15, B cols 16..31

            ibs, obs = {}, {}
            for gi, w in ((0, 8), (1, 8), (2, 16)):
                ibs[gi] = dram.tile([128, w], F32, name=f"ib{gi}")
                obs[gi] = dram.tile([128, w], F32, name=f"ob{gi}")

            for h in range(2):
                for q in range(NQ):
                    psums = []
                    for _pi in range(4):
                        pst = ps.tile([128, BS], F32, tag="ps",
                                      name=f"pst_{h}_{q}_{_pi}")
                        psums.append(pst)
                    for kt in range(NIT):
                        if (h, q) == (0, 0):
                            if kt < 4:
                                wsl = w00a0[:, kt, :]
                            elif kt < 8:
                                wsl = w00a1[:, kt - 4, :]
                            else:
                                wsl = w00b[:, kt - 8, :]
                        else:
                            wsl = wtiles[(h, q)][:, kt, :]
                        for ml in range(4):
                            nc.tensor.matmul(
                                psums[ml][:],
                                wsl[:, ml * 128:(ml + 1) * 128],
                                rhs[:, h * NIT + kt, :],
                                start=(kt == 0), stop=(kt == NIT - 1))
                    for ml in range(4):
                        m = q * 4 + ml
                        if h == 0:
                            nc.scalar.activation(y_big[:, m, :], psums[ml][:],
                                                 AF.Copy)
                        else:
                            nc.vector.scalar_tensor_tensor(
                                out=y_big[:, m, :], in0=psums[ml][:],
                                scalar=1.0, in1=y_big[:, m, :],
                                op0=OP.mult, op1=OP.add,
                                accum_out=stats[:, 8 * q + ml:8 * q + ml + 1])
                            dsc = drscr.tile([128, BS], F32, tag="drscr",
                                             name=f"dsc_{m}")
                            nc.scalar.activation(
                                dsc[:], y_big[:, m, :], AF.Square,
                                accum_out=stats[:, 8 * q + 4 + ml:
                                                8 * q + 5 + ml])
                    if h == 1 and q in (0, 1):
                        nc.sync.dma_start(ibs[q][:],
                                          stats[:, 8 * q:8 * q + 8])
                        nc.gpsimd.collective_compute(
                            "AllReduce", OP.add,
                            replica_groups=[list(range(NCORES))],
                            ins=[ibs[q].opt()], outs=[obs[q].opt()])
                    elif h == 1 and q == 3:
                        nc.sync.dma_start(ibs[2][:], stats[:, 16:32])
                        nc.gpsimd.collective_compute(
                            "AllReduce", OP.add,
                            replica_groups=[list(range(NCORES))],
                            ins=[ibs[2].opt()], outs=[obs[2].opt()])

            # ---- per-quarter finalize + normalize + store ----
            for q in range(NQ):
                if q in (0, 1):
                    nc.sync.dma_start(red[:, 8 * q:8 * q + 8], obs[q][:])
                elif q == 2:
                    nc.sync.dma_start(red[:, 16:32], obs[2][:])
                mean = small.tile([128, 4], F32, name=f"mean{q}")
                nc.vector.tensor_single_scalar(
                    out=mean[:], in_=red[:, 8 * q:8 * q + 4],
                    scalar=1.0 / B, op=OP.mult)
                msq = small.tile([128, 4], F32, name=f"msq{q}")
                nc.vector.tensor_single_scalar(
                    out=msq[:], in_=red[:, 8 * q + 4:8 * q + 8],
                    scalar=1.0 / B, op=OP.mult)
                var = small.tile([128, 4], F32, name=f"var{q}")
                nc.vector.tensor_tensor(var[:], mean[:], mean[:], op=OP.mult)
                nc.vector.tensor_tensor(var[:], msq[:], var[:], op=OP.subtract)
                stdt = small.tile([128, 4], F32, name=f"std{q}")
                nc.scalar.activation(stdt[:], var[:], AF.Sqrt, bias=epst[:])
                rstd = small.tile([128, 4], F32, name=f"rstd{q}")
                nc.vector.reciprocal(out=rstd[:], in_=stdt[:])
                acols = ab[:, 4 * q:4 * q + 4]
                bcols = ab[:, 16 + 4 * q:16 + 4 * q + 4]
                nc.vector.tensor_tensor(acols, gmt[:, 4 * q:4 * q + 4],
                                        rstd[:], op=OP.mult)
                nc.vector.tensor_tensor(bcols, mean[:], acols, op=OP.mult)
                nc.vector.tensor_tensor(bcols, btt[:, 4 * q:4 * q + 4],
                                        bcols, op=OP.subtract)

                if q < 3:
                    for half in range(2):
                        for k in range(2):
                            m = q * 4 + half * 2 + k
                            nc.vector.tensor_scalar(
                                out=y_big[:, m, :], in0=y_big[:, m, :],
                                scalar1=ab[:, m:m + 1],
                                scalar2=ab[:, 16 + m:17 + m],
                                op0=OP.mult, op1=OP.add)
                        m0 = q * 4 + half * 2
                        nc.sync.dma_start(yT_t[:, m0:m0 + 2, :],
                                          y_big[:, m0:m0 + 2, :])
                else:
                    for ml in range(4):
                        m = q * 4 + ml
                        nc.vector.tensor_scalar(
                            out=y_big[:, m, :], in0=y_big[:, m, :],
                            scalar1=ab[:, m:m + 1],
                            scalar2=ab[:, 16 + m:17 + m],
                            op0=OP.mult, op1=OP.add)
                        nc.sync.dma_start(yT_t[:, m:m + 1, :],
                                          y_big[:, m:m + 1, :])

    nc.compile()
    return nc


def _get_nc():
    if "nc" not in _CACHE:
        _CACHE["nc"] = _build_nc()
    return _CACHE["nc"]


def _fold(v):
    """(1, D) or (D,) feature vector -> (128, NIT) column-per-i-tile layout."""
    return np.ascontiguousarray(v.reshape(NIT, 128).T).astype(np.float32)


def kernel(x, scale, translate, wave_weight, base_weight, gamma, beta):
    x = np.asarray(x, dtype=np.float32)
    scale = np.asarray(scale, dtype=np.float32).reshape(1, D)
    translate = np.asarray(translate, dtype=np.float32).reshape(1, D)
    wave_weight = np.asarray(wave_weight, dtype=np.float32)
    base_weight = np.asarray(base_weight, dtype=np.float32)
    gamma = np.asarray(gamma, dtype=np.float32).reshape(D)
    beta = np.asarray(beta, dtype=np.float32).reshape(D)

    inv_s = 1.0 / np.maximum(scale, 1e-3)                     # (1, D)
    # t = x*s3 + b3 = phi/(2pi), phi = 3*(x - tr)*inv_s + pi/2
    s3 = 3.0 * inv_s / TWO_PI
    b3 = (math.pi / 2 - 3.0 * translate * inv_s) / TWO_PI
    # u^2 via Square(x*su + bu), u = (x - tr)*inv_s
    su = inv_s
    bu = -translate * inv_s

    wcat = np.concatenate([0.3 * base_weight.T,
                           (math.pi ** -0.25) * wave_weight.T], axis=0)
    wcat = np.ascontiguousarray(wcat.astype(np.float16))       # (2D, D)

    xT = np.ascontiguousarray(x.T.astype(np.float16))          # (D, B)

    common = dict(
        wT=wcat,
        s3=_fold(s3), b3=_fold(b3), su=_fold(su), bu=_fold(bu),
        gm=_fold(gamma), bt=_fold(beta),
    )
    in_maps = [
        dict(xT=np.ascontiguousarray(xT[:, c * BS:(c + 1) * BS]), **common)
        for c in range(NCORES)
    ]

    nc = _get_nc()
    res = run_bass_kernel_spmd(nc, in_maps, core_ids=list(range(NCORES)),
                               **_CACHE.pop("run_kwargs", {}))
    _CACHE["last_res"] = res
    yT = np.concatenate([res.results[c]["yT"] for c in range(NCORES)], axis=1)
    return np.ascontiguousarray(yT.T)


# revision 13
# speedup vs baseline: 1.1106x; 1.1106x over previous
"""BioWaveKAN fused kernel for 8 Trainium2 NeuronCores.

y = wavelet(x) @ (pi^-1/4 * Ww).T + x @ (0.3 * Wb).T   (single K=4096 contraction)
out = BatchNorm1d(y)  (training-mode batch stats, all-reduced across cores)

Sharding: data-parallel over batch (8 x 512 rows); BN stats via four 4KB
AllReduces, the first three hidden under the matmuls. Device layout is
transposed (features on partitions); host pre-transposes x and post-transposes
the output. Matmuls run fp16 x fp16 (fp32 PSUM accumulate); wavelet math runs
fp32 on ACT/DVE with an exact magic-number range reduction for cos(3u).

Structure per core:
  pass 1 (k-tiles 0..15  = x):       psum -> y_partial (ACT copy)
  pass 2 (k-tiles 16..31 = wavelet): y = y_partial + psum (DVE, fused sum(y))
                                     + DVE tensor_tensor_reduce (sum(y^2))
"""
import math

import numpy as np

from concourse import bacc
import concourse.tile as tile
import concourse.mybir as mybir
from concourse.bass_utils import run_bass_kernel_spmd

F32 = mybir.dt.float32
F16 = mybir.dt.float16
AF = mybir.ActivationFunctionType
OP = mybir.AluOpType

B = 4096          # batch
D = 2048          # in_dim == out_dim
NCORES = 8
BS = B // NCORES  # batch shard per core (512)
NIT = D // 128    # i-tiles (16)
NKT = 2 * NIT     # contraction tiles (32): 0..15 = x, 16..31 = wavelet
NOT = D // 128    # o-tiles (16)
NQ = 4            # quarters of o-tiles
BN_EPS = 1e-5
TWO_PI = 2.0 * math.pi
MAGIC = 1.5 * 2.0 ** 23

_CACHE = {}


def _build_nc():
    nc = bacc.Bacc()

    xT_d = nc.dram_tensor("xT", (D, BS), F16, kind="ExternalInput")
    wT_d = nc.dram_tensor("wT", (2 * D, D), F16, kind="ExternalInput")
    cst_d = nc.dram_tensor("cst", (128, 6 * NIT), F32, kind="ExternalInput")

    yT_d = nc.dram_tensor("yT", (D, BS), F32, kind="ExternalOutput")

    xT_t = xT_d[:].rearrange("(kt p) b -> p kt b", p=128)      # [128, 16, BS]
    wT_t = wT_d[:].rearrange("(kt p) o -> p kt o", p=128)      # [128, 32, D]
    yT_t = yT_d[:].rearrange("(mt p) b -> p mt b", p=128)      # [128, 16, BS]

    # stats column layout: quarter q holds cols [8q, 8q+8):
    #   [8q + ml]     = sum(y)   for o-tile m = 4q + ml
    #   [8q + 4 + ml] = sum(y^2)
    with tile.TileContext(nc) as tc:
        with (
            tc.tile_pool(name="big", bufs=1) as big,
            tc.tile_pool(name="small", bufs=1) as small,
            tc.tile_pool(name="wqh", bufs=2) as wqh,
            tc.tile_pool(name="wq", bufs=3) as wq,
            tc.tile_pool(name="scr", bufs=6) as scr,
            tc.tile_pool(name="escr", bufs=3) as escr,
            tc.tile_pool(name="drscr", bufs=3) as drscr,
            tc.tile_pool(name="ps", bufs=8, space="PSUM") as ps,
            tc.tile_pool(name="dram", bufs=1, space="DRAM") as dram,
        ):
            rhs = big.tile([128, NKT, BS], F16)

            # ---- DMA issue order: x chunk 0, first weight (split), consts,
            #      rest of x, remaining weights ----
            nc.sync.dma_start(rhs[:, 0:4, :], xT_t[:, 0:4, :])

            wtiles = {}
            w00a0 = wqh.tile([128, 4, 512], F16, tag="wqha", name="w00a0")
            nc.sync.dma_start(w00a0[:], wT_t[:, 0:4, 0:512])
            w00a1 = wqh.tile([128, 4, 512], F16, tag="wqha", name="w00a1")
            nc.sync.dma_start(w00a1[:], wT_t[:, 4:8, 0:512])
            w00b = wqh.tile([128, 8, 512], F16, tag="wqh", name="w00b")
            nc.sync.dma_start(w00b[:], wT_t[:, 8:16, 0:512])

            cstt = small.tile([128, 6 * NIT], F32)
            nc.sync.dma_start(cstt[:], cst_d[:])
            s3t = cstt[:, 0 * NIT:1 * NIT]
            b3t = cstt[:, 1 * NIT:2 * NIT]
            sut = cstt[:, 2 * NIT:3 * NIT]
            but = cstt[:, 3 * NIT:4 * NIT]
            gmt = cstt[:, 4 * NIT:5 * NIT]
            btt = cstt[:, 5 * NIT:6 * NIT]

            for c in range(1, 4):
                nc.sync.dma_start(rhs[:, c * 4:(c + 1) * 4, :],
                                  xT_t[:, c * 4:(c + 1) * 4, :])

            for h in range(2):
                for q in range(NQ):
                    if (h, q) == (0, 0):
                        continue
                    wt = wq.tile([128, NIT, 512], F16, tag="wq",
                                 name=f"w_{h}_{q}")
                    nc.sync.dma_start(
                        wt[:],
                        wT_t[:, h * NIT:(h + 1) * NIT, q * 512:(q + 1) * 512])
                    wtiles[(h, q)] = wt

            magict = small.tile([128, 1], F32)
            nc.vector.memset(magict[:], MAGIC)
            zbt = small.tile([128, 1], F32)
            nc.vector.memset(zbt[:], 0.0)
            epst = small.tile([128, 1], F32)
            nc.vector.memset(epst[:], BN_EPS)

            # ---- wavelet phase A: sin tiles (trig table set) ----
            # t/r on DVE, the magic-number round on GpSimd (parallel engines)
            for i in range(NIT):
                xf = rhs[:, i, :]
                tt = scr.tile([128, BS], F32, tag="scr", name=f"t_{i}")
                nc.vector.tensor_scalar(out=tt[:], in0=xf,
                                        scalar1=s3t[:, i:i + 1],
                                        scalar2=b3t[:, i:i + 1],
                                        op0=OP.mult, op1=OP.add)
                kt_ = scr.tile([128, BS], F32, tag="scr", name=f"k_{i}")
                nc.vector.tensor_scalar(out=kt_[:], in0=tt[:],
                                        scalar1=magict[:], scalar2=magict[:],
                                        op0=OP.add, op1=OP.subtract)
                rt = scr.tile([128, BS], F32, tag="scr", name=f"r_{i}")
                nc.vector.tensor_tensor(rt[:], tt[:], kt_[:], op=OP.subtract)
                nc.scalar.activation(rhs[:, NIT + i, :], rt[:], AF.Sin,
                                     bias=zbt[:], scale=TWO_PI)

            # ---- wavelet phase B: exp tiles (exp table set) ----
            for i in range(NIT):
                xf = rhs[:, i, :]
                qt = scr.tile([128, BS], F32, tag="scr", name=f"qq_{i}")
                nc.scalar.activation(qt[:], xf, AF.Square,
                                     bias=but[:, i:i + 1], scale=sut[:, i:i + 1])
                et = escr.tile([128, BS], F16, tag="escr", name=f"e_{i}")
                nc.scalar.activation(et[:], qt[:], AF.Exp,
                                     bias=zbt[:], scale=-0.5)
                nc.vector.tensor_tensor(rhs[:, NIT + i, :],
                                        rhs[:, NIT + i, :], et[:],
                                        op=OP.mult)

            # prefetch the sqrt table set (Copy lives in every set, so the h0
            # drains below don't reload; the finalize Sqrts are then free)
            sqpre = small.tile([128, 1], F32)
            nc.scalar.activation(sqpre[:], zbt[:], AF.Sqrt, bias=epst[:])

            # ---- matmuls + fused drains + per-quarter stats AllReduce ----
            y_big = big.tile([128, NOT, BS], F32)
            stats = small.tile([128, 2 * NOT], F32)
            red = small.tile([128, 2 * NOT], F32)
            ab = small.tile([128, 2 * NOT], F32)   # A cols 0..15, B cols 16..31

            ibs, obs = {}, {}
            for gi, w in ((0, 8), (1, 8), (2, 16)):
                ibs[gi] = dram.tile([128, w], F32, name=f"ib{gi}")
                obs[gi] = dram.tile([128, w], F32, name=f"ob{gi}")

            for h in range(2):
                for q in range(NQ):
                    psums = []
                    for _pi in range(4):
                        pst = ps.tile([128, BS], F32, tag="ps",
                                      name=f"pst_{h}_{q}_{_pi}")
                        psums.append(pst)
                    for kt in range(NIT):
                        if (h, q) == (0, 0):
                            if kt < 4:
                                wsl = w00a0[:, kt, :]
                            elif kt < 8:
                                wsl = w00a1[:, kt - 4, :]
                            else:
                                wsl = w00b[:, kt - 8, :]
                        else:
                            wsl = wtiles[(h, q)][:, kt, :]
                        for ml in range(4):
                            nc.tensor.matmul(
                                psums[ml][:],
                                wsl[:, ml * 128:(ml + 1) * 128],
                                rhs[:, h * NIT + kt, :],
                                start=(kt == 0), stop=(kt == NIT - 1))
                    for ml in range(4):
                        m = q * 4 + ml
                        if h == 0:
                            nc.scalar.activation(y_big[:, m, :], psums[ml][:],
                                                 AF.Copy)
                        else:
                            nc.vector.scalar_tensor_tensor(
                                out=y_big[:, m, :], in0=psums[ml][:],
                                scalar=1.0, in1=y_big[:, m, :],
                                op0=OP.mult, op1=OP.add,
                                accum_out=stats[:, 8 * q + ml:8 * q + ml + 1])
                            dsc = drscr.tile([128, BS], F32, tag="drscr",
                                             name=f"dsc_{m}")
                            nc.scalar.activation(
                                dsc[:], y_big[:, m, :], AF.Square,
                                accum_out=stats[:, 8 * q + 4 + ml:
                                                8 * q + 5 + ml])
                    if h == 1 and q in (0, 1):
                        nc.sync.dma_start(ibs[q][:],
                                          stats[:, 8 * q:8 * q + 8])
                        nc.gpsimd.collective_compute(
                            "AllReduce", OP.add,
                            replica_groups=[list(range(NCORES))],
                            ins=[ibs[q].opt()], outs=[obs[q].opt()])
                    elif h == 1 and q == 3:
                        nc.sync.dma_start(ibs[2][:], stats[:, 16:32])
                        nc.gpsimd.collective_compute(
                            "AllReduce", OP.add,
                            replica_groups=[list(range(NCORES))],
                            ins=[ibs[2].opt()], outs=[obs[2].opt()])

            # ---- per-quarter finalize + normalize + store ----
            for q in range(NQ):
                if q in (0, 1):
                    nc.sync.dma_start(red[:, 8 * q:8 * q + 8], obs[q][:])
                elif q == 2:
                    nc.sync.dma_start(red[:, 16:32], obs[2][:])
                mean = small.tile([128, 4], F32, name=f"mean{q}")
                nc.vector.tensor_single_scalar(
                    out=mean[:], in_=red[:, 8 * q:8 * q + 4],
                    scalar=1.0 / B, op=OP.mult)
                msq = small.tile([128, 4], F32, name=f"msq{q}")
                nc.vector.tensor_single_scalar(
                    out=msq[:], in_=red[:, 8 * q + 4:8 * q + 8],
                    scalar=1.0 / B, op=OP.mult)
                var = small.tile([128, 4], F32, name=f"var{q}")
                nc.vector.tensor_tensor(var[:], mean[:], mean[:], op=OP.mult)
                nc.vector.tensor_tensor(var[:], msq[:], var[:], op=OP.subtract)
                stdt = small.tile([128, 4], F32, name=f"std{q}")
                nc.scalar.activation(stdt[:], var[:], AF.Sqrt, bias=epst[:])
                rstd = small.tile([128, 4], F32, name=f"rstd{q}")
                nc.vector.reciprocal(out=rstd[:], in_=stdt[:])
                acols = ab[:, 4 * q:4 * q + 4]
                bcols = ab[:, 16 + 4 * q:16 + 4 * q + 4]
                nc.vector.tensor_tensor(acols, gmt[:, 4 * q:4 * q + 4],
                                        rstd[:], op=OP.mult)
                nc.vector.tensor_tensor(bcols, mean[:], acols, op=OP.mult)
                nc.vector.tensor_tensor(bcols, btt[:, 4 * q:4 * q + 4],
                                        bcols, op=OP.subtract)

                if q < 3:
                    for half in range(2):
                        for k in range(2):
                            m = q * 4 + half * 2 + k
                            nc.vector.tensor_scalar(
                                out=y_big[:, m, :], in0=y_big[:, m, :],
                                scalar1=ab[:, m:m + 1],
                                scalar2=ab[:, 16 + m:17 + m],
                                op0=OP.mult, op1=OP.add)
                        m0 = q * 4 + half * 2
                        nc.sync.dma_start(yT_t[:, m0:m0 + 2, :],
                                          y_big[:, m0:m0 + 2, :])
                else:
                    for ml in range(4):
                        m = q * 4 + ml
                        nc.vector.tensor_scalar(
                            out=y_big[:, m, :], in0=y_big[:, m, :],
                            scalar1=ab[:, m:m + 1],
                            scalar2=ab[:, 16 + m:17 + m],
                            op0=OP.mult, op1=OP.add)
                        nc.sync.dma_start(yT_t[:, m:m + 1, :],
                                          y_big[:, m:m + 1, :])

    nc.compile()
    return nc


def _get_nc():
    if "nc" not in _CACHE:
        _CACHE["nc"] = _build_nc()
    return _CACHE["nc"]


def _fold(v):
    """(1, D) or (D,) feature vector -> (128, NIT) column-per-i-tile layout."""
    return np.ascontiguousarray(v.reshape(NIT, 128).T).astype(np.float32)


def kernel(x, scale, translate, wave_weight, base_weight, gamma, beta):
    x = np.asarray(x, dtype=np.float32)
    scale = np.asarray(scale, dtype=np.float32).reshape(1, D)
    translate = np.asarray(translate, dtype=np.float32).reshape(1, D)
    wave_weight = np.asarray(wave_weight, dtype=np.float32)
    base_weight = np.asarray(base_weight, dtype=np.float32)
    gamma = np.asarray(gamma, dtype=np.float32).reshape(D)
    beta = np.asarray(beta, dtype=np.float32).reshape(D)

    inv_s = 1.0 / np.maximum(scale, 1e-3)                     # (1, D)
    # t = x*s3 + b3 = phi/(2pi), phi = 3*(x - tr)*inv_s + pi/2
    s3 = 3.0 * inv_s / TWO_PI
    b3 = (math.pi / 2 - 3.0 * translate * inv_s) / TWO_PI
    # u^2 via Square(x*su + bu), u = (x - tr)*inv_s
    su = inv_s
    bu = -translate * inv_s

    wcat = np.concatenate([0.3 * base_weight.T,
                           (math.pi ** -0.25) * wave_weight.T], axis=0)
    wcat = np.ascontiguousarray(wcat.astype(np.float16))       # (2D, D)

    xT = np.ascontiguousarray(x.T.astype(np.float16))          # (D, B)

    cst = np.concatenate([_fold(s3), _fold(b3), _fold(su), _fold(bu),
                          _fold(gamma), _fold(beta)], axis=1)
    common = dict(wT=wcat, cst=np.ascontiguousarray(cst))
    in_maps = [
        dict(xT=np.ascontiguousarray(xT[:, c * BS:(c + 1) * BS]), **common)
        for c in range(NCORES)
    ]

    nc = _get_nc()
    res = run_bass_kernel_spmd(nc, in_maps, core_ids=list(range(NCORES)),
                               **_CACHE.pop("run_kwargs", {}))
    _CACHE["last_res"] = res
    yT = np.concatenate([res.results[c]["yT"] for c in range(NCORES)], axis=1)
    return np.ascontiguousarray(yT.T)


# revision 14
# speedup vs baseline: 1.1465x; 1.0323x over previous
"""BioWaveKAN fused kernel for 8 Trainium2 NeuronCores.

y = wavelet(x) @ (pi^-1/4 * Ww).T + x @ (0.3 * Wb).T   (single K=4096 contraction)
out = BatchNorm1d(y)  (training-mode batch stats, all-reduced across cores)

Sharding: data-parallel over batch (8 x 512 rows); BN stats via four 4KB
AllReduces, the first three hidden under the matmuls. Device layout is
transposed (features on partitions); host pre-transposes x and post-transposes
the output. Matmuls run fp16 x fp16 (fp32 PSUM accumulate); wavelet math runs
fp32 on ACT/DVE with an exact magic-number range reduction for cos(3u).

Structure per core:
  pass 1 (k-tiles 0..15  = x):       psum -> y_partial (ACT copy)
  pass 2 (k-tiles 16..31 = wavelet): y = y_partial + psum (DVE, fused sum(y))
                                     + DVE tensor_tensor_reduce (sum(y^2))
"""
import math

import numpy as np

from concourse import bacc
import concourse.tile as tile
import concourse.mybir as mybir
from concourse.bass_utils import run_bass_kernel_spmd

F32 = mybir.dt.float32
F16 = mybir.dt.float16
AF = mybir.ActivationFunctionType
OP = mybir.AluOpType

B = 4096          # batch
D = 2048          # in_dim == out_dim
NCORES = 8
BS = B // NCORES  # batch shard per core (512)
NIT = D // 128    # i-tiles (16)
NKT = 2 * NIT     # contraction tiles (32): 0..15 = x, 16..31 = wavelet
NOT = D // 128    # o-tiles (16)
NQ = 4            # quarters of o-tiles
BN_EPS = 1e-5
TWO_PI = 2.0 * math.pi
MAGIC = 1.5 * 2.0 ** 23

_CACHE = {}


def _build_nc():
    nc = bacc.Bacc()

    xT_d = nc.dram_tensor("xT", (D, BS), F16, kind="ExternalInput")
    wT_d = nc.dram_tensor("wT", (2 * D, D), F16, kind="ExternalInput")
    cst_d = nc.dram_tensor("cst", (128, 6 * NIT), F32, kind="ExternalInput")

    yT_d = nc.dram_tensor("yT", (D, BS), F32, kind="ExternalOutput")

    xT_t = xT_d[:].rearrange("(kt p) b -> p kt b", p=128)      # [128, 16, BS]
    wT_t = wT_d[:].rearrange("(kt p) o -> p kt o", p=128)      # [128, 32, D]
    yT_t = yT_d[:].rearrange("(mt p) b -> p mt b", p=128)      # [128, 16, BS]

    # stats column layout: quarter q holds cols [8q, 8q+8):
    #   [8q + ml]     = sum(y)   for o-tile m = 4q + ml
    #   [8q + 4 + ml] = sum(y^2)
    with tile.TileContext(nc) as tc:
        with (
            tc.tile_pool(name="big", bufs=1) as big,
            tc.tile_pool(name="small", bufs=1) as small,
            tc.tile_pool(name="wqh", bufs=2) as wqh,
            tc.tile_pool(name="wq", bufs=3) as wq,
            tc.tile_pool(name="scr", bufs=6) as scr,
            tc.tile_pool(name="escr", bufs=3) as escr,
            tc.tile_pool(name="drscr", bufs=3) as drscr,
            tc.tile_pool(name="ps", bufs=8, space="PSUM") as ps,
            tc.tile_pool(name="dram", bufs=1, space="DRAM") as dram,
        ):
            rhs = big.tile([128, NKT, BS], F16)

            # ---- DMA issue order: x chunk 0, first weight (split), consts,
            #      rest of x, remaining weights ----
            nc.sync.dma_start(rhs[:, 0:4, :], xT_t[:, 0:4, :])

            wtiles = {}
            w00a = wqh.tile([128, 8, 512], F16, tag="wqh", name="w00a")
            nc.sync.dma_start(w00a[:], wT_t[:, 0:8, 0:512])
            w00b = wqh.tile([128, 8, 512], F16, tag="wqh", name="w00b")
            nc.sync.dma_start(w00b[:], wT_t[:, 8:16, 0:512])

            cstt = small.tile([128, 6 * NIT], F32)
            nc.sync.dma_start(cstt[:], cst_d[:])
            s3t = cstt[:, 0 * NIT:1 * NIT]
            b3t = cstt[:, 1 * NIT:2 * NIT]
            sut = cstt[:, 2 * NIT:3 * NIT]
            but = cstt[:, 3 * NIT:4 * NIT]
            gmt = cstt[:, 4 * NIT:5 * NIT]
            btt = cstt[:, 5 * NIT:6 * NIT]

            for c in range(1, 4):
                nc.sync.dma_start(rhs[:, c * 4:(c + 1) * 4, :],
                                  xT_t[:, c * 4:(c + 1) * 4, :])

            for h in range(2):
                for q in range(NQ):
                    if (h, q) == (0, 0):
                        continue
                    wt = wq.tile([128, NIT, 512], F16, tag="wq",
                                 name=f"w_{h}_{q}")
                    nc.sync.dma_start(
                        wt[:],
                        wT_t[:, h * NIT:(h + 1) * NIT, q * 512:(q + 1) * 512])
                    wtiles[(h, q)] = wt

            magict = small.tile([128, 1], F32)
            nc.vector.memset(magict[:], MAGIC)
            zbt = small.tile([128, 1], F32)
            nc.vector.memset(zbt[:], 0.0)
            epst = small.tile([128, 1], F32)
            nc.vector.memset(epst[:], BN_EPS)

            # ---- wavelet phase A: sin tiles (trig table set) ----
            # t/r on DVE, the magic-number round on GpSimd (parallel engines)
            for i in range(NIT):
                xf = rhs[:, i, :]
                tt = scr.tile([128, BS], F32, tag="scr", name=f"t_{i}")
                nc.vector.tensor_scalar(out=tt[:], in0=xf,
                                        scalar1=s3t[:, i:i + 1],
                                        scalar2=b3t[:, i:i + 1],
                                        op0=OP.mult, op1=OP.add)
                kt_ = scr.tile([128, BS], F32, tag="scr", name=f"k_{i}")
                nc.vector.tensor_scalar(out=kt_[:], in0=tt[:],
                                        scalar1=magict[:], scalar2=magict[:],
                                        op0=OP.add, op1=OP.subtract)
                rt = scr.tile([128, BS], F32, tag="scr", name=f"r_{i}")
                nc.vector.tensor_tensor(rt[:], tt[:], kt_[:], op=OP.subtract)
                nc.scalar.activation(rhs[:, NIT + i, :], rt[:], AF.Sin,
                                     bias=zbt[:], scale=TWO_PI)

            # ---- wavelet phase B: exp tiles (exp table set) ----
            for i in range(NIT):
                xf = rhs[:, i, :]
                qt = scr.tile([128, BS], F32, tag="scr", name=f"qq_{i}")
                nc.scalar.activation(qt[:], xf, AF.Square,
                                     bias=but[:, i:i + 1], scale=sut[:, i:i + 1])
                et = escr.tile([128, BS], F16, tag="escr", name=f"e_{i}")
                nc.scalar.activation(et[:], qt[:], AF.Exp,
                                     bias=zbt[:], scale=-0.5)
                nc.vector.tensor_tensor(rhs[:, NIT + i, :],
                                        rhs[:, NIT + i, :], et[:],
                                        op=OP.mult)

            # prefetch the sqrt table set (Copy lives in every set, so the h0
            # drains below don't reload; the finalize Sqrts are then free)
            sqpre = small.tile([128, 1], F32)
            nc.scalar.activation(sqpre[:], zbt[:], AF.Sqrt, bias=epst[:])

            # ---- matmuls + fused drains + per-quarter stats AllReduce ----
            y_big = big.tile([128, NOT, BS], F32)
            stats = small.tile([128, 2 * NOT], F32)
            red = small.tile([128, 2 * NOT], F32)
            ab = small.tile([128, 2 * NOT], F32)   # A cols 0..15, B cols 16..31

            ibs, obs = {}, {}
            for gi, w in ((0, 8), (1, 8), (2, 16)):
                ibs[gi] = dram.tile([128, w], F32, name=f"ib{gi}")
                obs[gi] = dram.tile([128, w], F32, name=f"ob{gi}")

            for h in range(2):
                for q in range(NQ):
                    psums = []
                    for _pi in range(4):
                        pst = ps.tile([128, BS], F32, tag="ps",
                                      name=f"pst_{h}_{q}_{_pi}")
                        psums.append(pst)
                    for kt in range(NIT):
                        if (h, q) == (0, 0):
                            wsl = (w00a[:, kt, :] if kt < 8
                                   else w00b[:, kt - 8, :])
                        else:
                            wsl = wtiles[(h, q)][:, kt, :]
                        for ml in range(4):
                            nc.tensor.matmul(
                                psums[ml][:],
                                wsl[:, ml * 128:(ml + 1) * 128],
                                rhs[:, h * NIT + kt, :],
                                start=(kt == 0), stop=(kt == NIT - 1))
                    for ml in range(4):
                        m = q * 4 + ml
                        if h == 0:
                            nc.scalar.activation(y_big[:, m, :], psums[ml][:],
                                                 AF.Copy)
                        else:
                            nc.vector.scalar_tensor_tensor(
                                out=y_big[:, m, :], in0=psums[ml][:],
                                scalar=1.0, in1=y_big[:, m, :],
                                op0=OP.mult, op1=OP.add,
                                accum_out=stats[:, 8 * q + ml:8 * q + ml + 1])
                            dsc = drscr.tile([128, BS], F32, tag="drscr",
                                             name=f"dsc_{m}")
                            nc.scalar.activation(
                                dsc[:], y_big[:, m, :], AF.Square,
                                accum_out=stats[:, 8 * q + 4 + ml:
                                                8 * q + 5 + ml])
                    if h == 1 and q in (0, 1):
                        nc.sync.dma_start(ibs[q][:],
                                          stats[:, 8 * q:8 * q + 8])
                        nc.gpsimd.collective_compute(
                            "AllReduce", OP.add,
                            replica_groups=[list(range(NCORES))],
                            ins=[ibs[q].opt()], outs=[obs[q].opt()])
                    elif h == 1 and q == 3:
                        nc.sync.dma_start(ibs[2][:], stats[:, 16:32])
                        nc.gpsimd.collective_compute(
                            "AllReduce", OP.add,
                            replica_groups=[list(range(NCORES))],
                            ins=[ibs[2].opt()], outs=[obs[2].opt()])

            # ---- per-quarter finalize + normalize + store ----
            for q in range(NQ):
                if q in (0, 1):
                    nc.sync.dma_start(red[:, 8 * q:8 * q + 8], obs[q][:])
                elif q == 2:
                    nc.sync.dma_start(red[:, 16:32], obs[2][:])
                mean = small.tile([128, 4], F32, name=f"mean{q}")
                nc.vector.tensor_single_scalar(
                    out=mean[:], in_=red[:, 8 * q:8 * q + 4],
                    scalar=1.0 / B, op=OP.mult)
                msq = small.tile([128, 4], F32, name=f"msq{q}")
                nc.vector.tensor_single_scalar(
                    out=msq[:], in_=red[:, 8 * q + 4:8 * q + 8],
                    scalar=1.0 / B, op=OP.mult)
                var = small.tile([128, 4], F32, name=f"var{q}")
                nc.vector.tensor_tensor(var[:], mean[:], mean[:], op=OP.mult)
                nc.vector.tensor_tensor(var[:], msq[:], var[:], op=OP.subtract)
                stdt = small.tile([128, 4], F32, name=f"std{q}")
                nc.scalar.activation(stdt[:], var[:], AF.Sqrt, bias=epst[:])
                rstd = small.tile([128, 4], F32, name=f"rstd{q}")
                nc.vector.reciprocal(out=rstd[:], in_=stdt[:])
                acols = ab[:, 4 * q:4 * q + 4]
                bcols = ab[:, 16 + 4 * q:16 + 4 * q + 4]
                nc.vector.tensor_tensor(acols, gmt[:, 4 * q:4 * q + 4],
                                        rstd[:], op=OP.mult)
                nc.vector.tensor_tensor(bcols, mean[:], acols, op=OP.mult)
                nc.vector.tensor_tensor(bcols, btt[:, 4 * q:4 * q + 4],
                                        bcols, op=OP.subtract)

                if q < 3:
                    for half in range(2):
                        for k in range(2):
                            m = q * 4 + half * 2 + k
                            nc.vector.tensor_scalar(
                                out=y_big[:, m, :], in0=y_big[:, m, :],
                                scalar1=ab[:, m:m + 1],
                                scalar2=ab[:, 16 + m:17 + m],
                                op0=OP.mult, op1=OP.add)
                        m0 = q * 4 + half * 2
                        nc.sync.dma_start(yT_t[:, m0:m0 + 2, :],
                                          y_big[:, m0:m0 + 2, :])
                else:
                    for ml in range(4):
                        m = q * 4 + ml
                        nc.vector.tensor_scalar(
                            out=y_big[:, m, :], in0=y_big[:, m, :],
                            scalar1=ab[:, m:m + 1],
                            scalar2=ab[:, 16 + m:17 + m],
                            op0=OP.mult, op1=OP.add)
                        nc.sync.dma_start(yT_t[:, m:m + 1, :],
                                          y_big[:, m:m + 1, :])

    nc.compile()
    return nc


def _get_nc():
    if "nc" not in _CACHE:
        _CACHE["nc"] = _build_nc()
    return _CACHE["nc"]


def _fold(v):
    """(1, D) or (D,) feature vector -> (128, NIT) column-per-i-tile layout."""
    return np.ascontiguousarray(v.reshape(NIT, 128).T).astype(np.float32)


def kernel(x, scale, translate, wave_weight, base_weight, gamma, beta):
    x = np.asarray(x, dtype=np.float32)
    scale = np.asarray(scale, dtype=np.float32).reshape(1, D)
    translate = np.asarray(translate, dtype=np.float32).reshape(1, D)
    wave_weight = np.asarray(wave_weight, dtype=np.float32)
    base_weight = np.asarray(base_weight, dtype=np.float32)
    gamma = np.asarray(gamma, dtype=np.float32).reshape(D)
    beta = np.asarray(beta, dtype=np.float32).reshape(D)

    inv_s = 1.0 / np.maximum(scale, 1e-3)                     # (1, D)
    # t = x*s3 + b3 = phi/(2pi), phi = 3*(x - tr)*inv_s + pi/2
    s3 = 3.0 * inv_s / TWO_PI
    b3 = (math.pi / 2 - 3.0 * translate * inv_s) / TWO_PI
    # u^2 via Square(x*su + bu), u = (x - tr)*inv_s
    su = inv_s
    bu = -translate * inv_s

    wcat = np.concatenate([0.3 * base_weight.T,
                           (math.pi ** -0.25) * wave_weight.T], axis=0)
    wcat = np.ascontiguousarray(wcat.astype(np.float16))       # (2D, D)

    xT = np.ascontiguousarray(x.T.astype(np.float16))          # (D, B)

    cst = np.concatenate([_fold(s3), _fold(b3), _fold(su), _fold(bu),
                          _fold(gamma), _fold(beta)], axis=1)
    common = dict(wT=wcat, cst=np.ascontiguousarray(cst))
    in_maps = [
        dict(xT=np.ascontiguousarray(xT[:, c * BS:(c + 1) * BS]), **common)
        for c in range(NCORES)
    ]

    nc = _get_nc()
    res = run_bass_kernel_spmd(nc, in_maps, core_ids=list(range(NCORES)),
                               **_CACHE.pop("run_kwargs", {}))
    _CACHE["last_res"] = res
    yT = np.concatenate([res.results[c]["yT"] for c in range(NCORES)], axis=1)
    return np.ascontiguousarray(yT.T)
